# revision 7
# baseline (speedup 1.0000x reference)
"""Trainium2 kernel for nn_ContrasiveLoss (segment-reduce contrastive loss).

Strategy (data-parallel, one image per NeuronCore, 8 cores):
  Per-image loss needs only per-segment statistics
      counts[k], sums[k, c], sqsums[k, c]
  (the variance term telescopes).  Statistics are computed as one-hot
  matmuls on the TensorEngine in fp8-e4m3 DoubleRow mode: each matmul
  contracts 256 pixels (2 k-tiles of 128 partitions) for 8 pixel groups
  at once (8 groups x 16 labels = 128 PSUM partitions).  Per 256-pixel
  super-window there are two accumulating matmuls:
      A: one-hot^T @ features            -> [128, 256]  (bank A)
      B: one-hot^T @ [features^2 | 1]    -> [128, 257]  (bank B)
  Features and the one-hot encoding of the labels are marshaled host-side
  into fp8 with pixels on partitions, so device DMAs are plain contiguous
  copies (no xbar transpose).  Squares are computed on-device (DVE + ACT
  split).  A small epilogue folds the 8 group blocks, computes the
  variance/hinge/regularizer terms and writes one scalar; the host sums
  the 8 scalars and divides by (B+1).
"""

import ml_dtypes
import numpy as np

import concourse.bass as bass
import concourse.mybir as mybir
import concourse.tile as tile
from concourse.bass_utils import run_bass_kernel_spmd
from concourse.vector_clock import ScopedClock

# ---------------------------------------------------------------- problem dims
B, C, H, W = 8, 32, 512, 512
K = 16
G = 8                    # pixel groups; G*K = 128 PSUM partitions
N = H * W                # pixels per image
PG = N // G              # 32768 pixels per group
SW = PG // 256           # 128 super-windows (256 pixels each, per group)
CHUNK = 8                # super-windows per DMA chunk
NCHUNK = SW // CHUNK     # 16
FC = 2 * G * C           # 512 feature cols per super-window
OC = 2 * G * K           # 256 one-hot cols per super-window
SQS = G * C + 1          # 257: squares + ones column (per k-tile)
# squares column split across engines (of the 256 feature columns)
DVE_COLS = 120
ACT_COLS = 136
POOL_COLS = 256 - DVE_COLS - ACT_COLS

DD = 2.5
GAMMA = 0.005

FP8 = mybir.dt.float8e4
FP8_NP = ml_dtypes.float8_e4m3
FP32 = mybir.dt.float32

TRACE = False            # test harness flips this for NTFF profiling
DEBUG_STATS = False      # also emit the raw [128, 513] stats for verification


# ------------------------------------------------- container-specific patches
def _patch_tile_drain() -> None:
    """This container's walrus build accepts only ONE sync-wait command per
    instruction, but TileContext's tail drain attaches one wait per active
    semaphore lane.  Split the tail drain into a chain of single-wait drains.
    """
    if getattr(tile.TileContext, "_drain_split_patched", False):
        return

    def _drain_and_barrier(self, tick_clock, wait_clock):
        drain_inst = self.nc.sync.drain()
        wait_clock.add_sem_waits(
            drain_inst.ins, ScopedClock({None: tick_clock.global_clock})
        )
        si = drain_inst.ins.sync_info
        if si is not None and len(si.on_wait) > 1:
            waits = list(si.on_wait)
            drain_inst.ins.sync_info = mybir.SyncInfo(
                on_wait=[waits[0]], on_update=list(si.on_update)
            )
            for w in waits[1:]:
                d2 = self.nc.sync.drain()
                d2.ins.sync_info = mybir.SyncInfo(on_wait=[w], on_update=[])

        self.nc.all_engine_barrier()
        assert self.sems is not None
        popped = self.nc._tile_sem_poison_stack.pop()
        assert popped is self._sem_poison
        self.nc.clear_and_free_semaphores(list(self.sems.allocated().values()))
        self.nc.all_engine_barrier()

    tile.TileContext._drain_and_barrier = _drain_and_barrier
    tile.TileContext._drain_split_patched = True


def _split_multi_waits(nc) -> None:
    """Walrus accepts one sync-wait per instruction: hoist extra waits onto
    single-wait Drain instructions on the same engine, inserted just before."""
    for fn in nc.m.functions:
        for blk in fn.blocks:
            changed = False
            out = []
            for ins in blk.instructions:
                si = ins.sync_info
                if si is not None and len(si.on_wait) > 1:
                    changed = True
                    waits = list(si.on_wait)
                    for j, w in enumerate(waits[:-1]):
                        d = mybir.InstDrain(name=f"{ins.name}-ws{j}")
                        d.engine = ins.engine
                        d.sync_info = mybir.SyncInfo(on_wait=[w], on_update=[])
                        out.append(d)
                    ins.sync_info = mybir.SyncInfo(
                        on_wait=[waits[-1]], on_update=list(si.on_update)
                    )
                out.append(ins)
            if changed:
                blk.instructions = out


# ------------------------------------------------------------- device program
def _host_constants():
    # stats row r = g*16+k; cols: [sums (g',c) 0:256 | sqs (g',c) 256:512 |
    # counts 512].  Keep only the block-diagonal g'==g pieces + counts.
    mask = np.zeros((128, 513), dtype=np.float32)
    for r in range(128):
        g = r // K
        mask[r, g * C:(g + 1) * C] = 1.0
        mask[r, 256 + g * C:256 + (g + 1) * C] = 1.0
        mask[r, 512] = 1.0
    sel = np.zeros((128, K), dtype=np.float32)
    for r in range(128):
        sel[r, r % K] = 1.0
    ident16 = np.eye(16, dtype=np.float32)
    ones_row = np.ones((1, 16), dtype=np.float32)
    # final-combine column: divides the per-label partial losses by K
    ones_col = np.full((16, 1), 1.0 / K, dtype=np.float32)
    # pair mask pre-scaled by the hinge-term 1/(K-1) normalizer
    triu = np.triu(np.ones((K, K), dtype=np.float32), k=1) / (K - 1)
    return mask, sel, ident16, ones_row, ones_col, triu


def _build_kernel():
    _patch_tile_drain()
    nc = bass.Bass("TRN2")

    fpk = nc.dram_tensor("fpk", [128, SW * FC], FP8, kind="ExternalInput")
    ohd = nc.dram_tensor("ohd", [128, SW * OC], FP8, kind="ExternalInput")
    out = nc.dram_tensor("out", [1, 1], FP32, kind="ExternalOutput")
    dbg = (nc.dram_tensor("dbg", [128, 513], FP32, kind="ExternalOutput")
           if DEBUG_STATS else None)

    mask_np, sel_np, id16_np, ones_row_np, ones_col_np, triu_np = \
        _host_constants()
    c_mask = nc.inline_tensor(mask_np, name="c_mask")
    c_sel = nc.inline_tensor(sel_np, name="c_sel")
    c_id16 = nc.inline_tensor(id16_np, name="c_id16")
    c_ones_row = nc.inline_tensor(ones_row_np, name="c_ones_row")
    c_ones_col = nc.inline_tensor(ones_col_np, name="c_ones_col")
    c_triu = nc.inline_tensor(triu_np, name="c_triu")

    DR = mybir.MatmulPerfMode.DoubleRow

    with tile.TileContext(nc) as tc:
        with (
            tc.tile_pool(name="consts", bufs=1) as consts,
            tc.tile_pool(name="feat", bufs=3) as featp,
            tc.tile_pool(name="oh", bufs=3) as ohp,
            tc.tile_pool(name="sq", bufs=3) as sqp,
            tc.tile_pool(name="acc", bufs=1, space="PSUM") as accp,
            tc.tile_pool(name="eps", bufs=1, space="PSUM") as epsp,
            tc.tile_pool(name="epi", bufs=1) as epi,
        ):
            psA = accp.tile([128, 256], FP32)   # one-hot @ features
            psB = accp.tile([128, 257], FP32)   # one-hot @ [features^2 | 1]

            for ci in range(NCHUNK):
                ft = featp.tile([128, CHUNK * FC], FP8)
                nc.sync.dma_start(
                    out=ft, in_=fpk[:, ci * CHUNK * FC:(ci + 1) * CHUNK * FC]
                )
                oh = ohp.tile([128, CHUNK * OC], FP8)
                nc.gpsimd.dma_start(
                    out=oh, in_=ohd[:, ci * CHUNK * OC:(ci + 1) * CHUNK * OC]
                )
                sq = sqp.tile([128, CHUNK * 2 * SQS], FP8)

                ft4 = ft.rearrange("p (w i j) -> p w i j", i=2, j=G * C)
                sq4 = sq.rearrange("p (w i s) -> p w i s", i=2, s=SQS)
                oh4 = oh.rearrange("p (w i m) -> p w i m", i=2, m=G * K)

                # squares: column-split across DVE / ACT (/ Pool)
                c1 = DVE_COLS
                c2 = DVE_COLS + ACT_COLS
                nc.vector.tensor_mul(
                    sq4[:, :, :, 0:c1], ft4[:, :, :, 0:c1], ft4[:, :, :, 0:c1]
                )
                nc.scalar.activation(
                    out=sq4[:, :, :, c1:c2], in_=ft4[:, :, :, c1:c2],
                    func=mybir.ActivationFunctionType.Square,
                )
                if POOL_COLS:
                    nc.gpsimd.tensor_mul(
                        sq4[:, :, :, c2:G * C],
                        ft4[:, :, :, c2:G * C], ft4[:, :, :, c2:G * C],
                    )
                nc.vector.memset(sq4[:, :, :, G * C:SQS], 1.0)

                # ---- segment matmuls (DoubleRow: 256-pixel contraction)
                for w in range(CHUNK):
                    gw = ci * CHUNK + w
                    lhsT = oh4[:, w]
                    nc.tensor.matmul(
                        psA[:, :], lhsT, ft4[:, w],
                        start=(gw == 0), stop=(gw == SW - 1), perf_mode=DR,
                    )
                    nc.tensor.matmul(
                        psB[:, :], lhsT, sq4[:, w],
                        start=(gw == 0), stop=(gw == SW - 1), perf_mode=DR,
                    )

            # ---- constants into SBUF (issued after the streaming DMAs so
            # they don't delay the first feature chunk; only the epilogue
            # consumes them)
            sb_mask = consts.tile([128, 513], FP32)
            nc.sync.dma_start(out=sb_mask, in_=c_mask[:, :])
            sb_sel = consts.tile([128, K], FP32)
            nc.sync.dma_start(out=sb_sel, in_=c_sel[:, :])
            sb_id16 = consts.tile([16, 16], FP32)
            nc.sync.dma_start(out=sb_id16, in_=c_id16[:, :])
            sb_ones_row = consts.tile([1, 16], FP32)
            nc.sync.dma_start(out=sb_ones_row, in_=c_ones_row[:, :])
            sb_ones_col = consts.tile([16, 1], FP32)
            nc.sync.dma_start(out=sb_ones_col, in_=c_ones_col[:, :])
            sb_triu = consts.tile([16, 16], FP32)
            nc.sync.dma_start(out=sb_triu, in_=c_triu[:, :])

            # ================= epilogue: stats -> scalar loss =================
            if dbg is not None:
                stats = epi.tile([128, 513], FP32)
                nc.vector.tensor_copy(stats[:, 0:256], psA)
                nc.vector.tensor_copy(stats[:, 256:513], psB)
                nc.sync.dma_start(out=dbg[:, :], in_=stats)

            masked = epi.tile([128, 513], FP32)
            nc.vector.tensor_mul(masked[:, 0:256], psA, sb_mask[:, 0:256])
            nc.vector.tensor_mul(masked[:, 256:513], psB, sb_mask[:, 256:513])

            # fold the 8 group blocks into [16, *] with sel (r -> r%16)
            psum2a = epsp.tile([16, 256], FP32)
            nc.tensor.matmul(psum2a[:, :], sb_sel, masked[:, 0:256],
                             start=True, stop=True)
            psum2b = epsp.tile([16, 257], FP32)
            nc.tensor.matmul(psum2b[:, :], sb_sel, masked[:, 256:513],
                             start=True, stop=True)

            # fold the 8 (g', c) column blocks of 32 down to [16, 32]
            # (DVE may read at most one non-scalar input from PSUM)
            comb_a = epi.tile([16, 128], FP32)
            nc.vector.tensor_copy(comb_a, psum2a[:, 0:128])
            t128 = epi.tile([16, 128], FP32)
            nc.vector.tensor_add(t128, comb_a, psum2a[:, 128:256])
            t64 = epi.tile([16, 64], FP32)
            nc.vector.tensor_add(t64, t128[:, 0:64], t128[:, 64:128])
            sums = epi.tile([16, 32], FP32)
            nc.vector.tensor_add(sums, t64[:, 0:32], t64[:, 32:64])
            comb_b = epi.tile([16, 128], FP32)
            nc.vector.tensor_copy(comb_b, psum2b[:, 0:128])
            u128 = epi.tile([16, 128], FP32)
            nc.vector.tensor_add(u128, comb_b, psum2b[:, 128:256])
            u64 = epi.tile([16, 64], FP32)
            nc.vector.tensor_add(u64, u128[:, 0:64], u128[:, 64:128])
            sqs = epi.tile([16, 32], FP32)
            nc.vector.tensor_add(sqs, u64[:, 0:32], u64[:, 32:64])

            recip = epi.tile([16, 1], FP32)
            nc.vector.reciprocal(out=recip, in_=psum2b[:, 256:257])

            means = epi.tile([16, 32], FP32)
            nc.vector.tensor_scalar_mul(out=means, in0=sums, scalar1=recip)
            msq = epi.tile([16, 32], FP32)
            nc.vector.tensor_mul(msq, means, means)
            m2 = epi.tile([16, 1], FP32)
            nc.vector.tensor_reduce(
                out=m2, in_=msq, axis=mybir.AxisListType.X,
                op=mybir.AluOpType.add,
            )
            sqk = epi.tile([16, 1], FP32)
            nc.vector.tensor_reduce(
                out=sqk, in_=sqs, axis=mybir.AxisListType.X,
                op=mybir.AluOpType.add,
            )
            # vark = sqk/counts - m2 in one op
            vark = epi.tile([16, 1], FP32)
            nc.vector.tensor_scalar(
                out=vark, in0=sqk, scalar1=recip, scalar2=m2,
                op0=mybir.AluOpType.mult, op1=mybir.AluOpType.subtract,
            )

            # pairwise distances: diff2 = m2_i + m2_j - 2 * means @ means.T
            psumT = epsp.tile([32, 16], FP32)
            nc.tensor.transpose(psumT[:, :], means, sb_id16)
            meansT = epi.tile([32, 16], FP32)
            nc.vector.tensor_copy(meansT, psumT)
            meansTn2 = epi.tile([32, 16], FP32)
            nc.vector.tensor_scalar_mul(out=meansTn2, in0=meansT, scalar1=-2.0)

            psumR = epsp.tile([1, 16], FP32)
            nc.tensor.transpose(psumR[:, :], m2, sb_id16)
            m2row = epi.tile([1, 16], FP32)
            nc.vector.tensor_copy(m2row, psumR)

            psumD = epsp.tile([16, 16], FP32)
            nc.tensor.matmul(psumD[:, :], sb_ones_row, m2row,
                             start=True, stop=False)
            nc.tensor.matmul(psumD[:, :], m2row, sb_ones_row,
                             start=False, stop=False)
            nc.tensor.matmul(psumD[:, :], meansTn2, meansT,
                             start=False, stop=True)

            # one ACT sqrt over [diff2 | m2] -> [dist | reg]
            dm = epi.tile([16, 17], FP32)
            nc.vector.tensor_scalar_max(out=dm[:, 0:16], in0=psumD,
                                        scalar1=0.0)
            nc.vector.tensor_copy(dm[:, 16:17], m2)
            dr = epi.tile([16, 17], FP32)
            nc.scalar.activation(out=dr, in_=dm,
                                 func=mybir.ActivationFunctionType.Sqrt)

            hinge = epi.tile([16, 16], FP32)
            nc.vector.tensor_scalar(
                out=hinge, in0=dr[:, 0:16], scalar1=-1.0, scalar2=2.0 * DD,
                op0=mybir.AluOpType.mult, op1=mybir.AluOpType.add,
            )
            nc.vector.tensor_scalar_max(out=hinge, in0=hinge, scalar1=0.0)
            nc.vector.tensor_mul(hinge, hinge, hinge)

            # final [16, 18] = [vark | gamma*reg | hinge * triu/(K-1)];
            # ones_col is pre-scaled by 1/K, so loss = sum(fin)
            final = epi.tile([16, 18], FP32)
            nc.vector.tensor_copy(final[:, 0:1], vark)
            nc.vector.tensor_scalar(
                out=final[:, 1:2], in0=dr[:, 16:17], scalar1=GAMMA,
                scalar2=None, op0=mybir.AluOpType.mult,
            )
            nc.vector.tensor_mul(final[:, 2:18], hinge, sb_triu)

            psumS = epsp.tile([1, 18], FP32)
            nc.tensor.matmul(psumS[:, :], sb_ones_col, final,
                             start=True, stop=True)
            loss = epi.tile([1, 1], FP32)
            nc.vector.tensor_reduce(
                out=loss, in_=psumS, axis=mybir.AxisListType.X,
                op=mybir.AluOpType.add,
            )
            nc.sync.dma_start(out=out[:, :], in_=loss)

    _split_multi_waits(nc)
    return nc


_NC_CACHE = {}


def _get_kernel():
    key = (DEBUG_STATS,)
    if key not in _NC_CACHE:
        _NC_CACHE[key] = _build_kernel()
    return _NC_CACHE[key]


# --------------------------------------------------------------- entry point
def _marshal_image(feat: np.ndarray, lab: np.ndarray):
    # feat [C, H, W] f32 -> fpk [128 p, (w i g c)] fp8;
    # lab [H, W] int -> one-hot ohd [128 p, (w i g k)] fp8.
    # pixel n = g*PG + w*256 + i*128 + p
    f5 = feat.reshape(C, G, SW, 2, 128)
    fpk = np.ascontiguousarray(
        f5.transpose(4, 2, 3, 1, 0).reshape(128, SW * FC)
    ).astype(FP8_NP)
    l4 = lab.reshape(G, SW, 2, 128)
    ohb = (l4[..., None] == np.arange(K, dtype=l4.dtype))
    ohd = np.ascontiguousarray(
        ohb.transpose(3, 1, 2, 0, 4).reshape(128, SW * OC)
    ).astype(FP8_NP)
    return fpk, ohd


def kernel(features_batch, labels_batch, num_instances):
    assert int(num_instances) == K
    features_batch = np.asarray(features_batch, dtype=np.float32)
    labels_batch = np.asarray(labels_batch)
    assert features_batch.shape == (B, C, H, W)

    nc = _get_kernel()
    in_maps = []
    for i in range(B):
        fpk, ohd = _marshal_image(features_batch[i], labels_batch[i])
        in_maps.append({"fpk": fpk, "ohd": ohd})

    res = run_bass_kernel_spmd(
        nc, in_maps, core_ids=list(range(B)), trace=TRACE
    )
    kernel.last_result = res
    losses = [res.results[i]["out"][0, 0] for i in range(B)]
    total = np.float64(0.0)
    for v in losses:
        total += np.float64(v)
    return np.array(total / (B + 1), dtype=np.float32)


# revision 10
# speedup vs baseline: 1.2744x; 1.2744x over previous
"""Trainium2 kernel for nn_ContrasiveLoss (segment-reduce contrastive loss).

Strategy (data-parallel, one image per NeuronCore, 8 cores):
  Per-image loss needs only per-segment statistics
      counts[k], sums[k, c], sqsums[k, c]
  (the variance term telescopes).  Statistics are computed as one-hot
  matmuls on the TensorEngine in fp8-e4m3 DoubleRow mode: each matmul
  contracts 256 pixels (2 k-tiles of 128 partitions) for 8 pixel groups
  at once (8 groups x 16 labels = 128 PSUM partitions).  Per 256-pixel
  super-window there are two accumulating matmuls:
      A: one-hot^T @ features            -> [128, 256]  (bank A)
      B: one-hot^T @ [features^2 | 1]    -> [128, 257]  (bank B)
  Features and the one-hot encoding of the labels are marshaled host-side
  into fp8 with pixels on partitions, so device DMAs are plain contiguous
  copies (no xbar transpose).  Squares are computed on-device (DVE + ACT
  split).  A small epilogue folds the 8 group blocks, computes the
  variance/hinge/regularizer terms and writes one scalar; the host sums
  the 8 scalars and divides by (B+1).
"""

import ml_dtypes
import numpy as np

import concourse.bass as bass
import concourse.mybir as mybir
import concourse.tile as tile
from concourse.bass_utils import run_bass_kernel_spmd
from concourse.vector_clock import ScopedClock

# ---------------------------------------------------------------- problem dims
B, C, H, W = 8, 32, 512, 512
K = 16
G = 8                    # pixel groups; G*K = 128 PSUM partitions
N = H * W                # pixels per image
PG = N // G              # 32768 pixels per group
SW = PG // 256           # 128 super-windows (256 pixels each, per group)
CHUNK = 16               # super-windows per DMA chunk
NCHUNK = SW // CHUNK     # 8
FC = 2 * G * C           # 512 feature cols per super-window
OC = 2 * G * K           # 256 one-hot cols per super-window
SQS = G * C + 1          # 257: squares + ones column (per k-tile)
# squares column split across engines (of the 256 feature columns)
DVE_COLS = 132
ACT_COLS = 124
POOL_COLS = 256 - DVE_COLS - ACT_COLS
BUFS = 5                 # chunk pipeline depth

DD = 2.5
GAMMA = 0.005

FP8 = mybir.dt.float8e4
FP8_NP = ml_dtypes.float8_e4m3
FP32 = mybir.dt.float32

TRACE = False            # test harness flips this for NTFF profiling
DEBUG_STATS = False      # also emit the raw [128, 513] stats for verification


# ------------------------------------------------- container-specific patches
def _patch_tile_drain() -> None:
    """This container's walrus build accepts only ONE sync-wait command per
    instruction, but TileContext's tail drain attaches one wait per active
    semaphore lane.  Split the tail drain into a chain of single-wait drains.
    """
    if getattr(tile.TileContext, "_drain_split_patched", False):
        return

    def _drain_and_barrier(self, tick_clock, wait_clock):
        drain_inst = self.nc.sync.drain()
        wait_clock.add_sem_waits(
            drain_inst.ins, ScopedClock({None: tick_clock.global_clock})
        )
        si = drain_inst.ins.sync_info
        if si is not None and len(si.on_wait) > 1:
            waits = list(si.on_wait)
            drain_inst.ins.sync_info = mybir.SyncInfo(
                on_wait=[waits[0]], on_update=list(si.on_update)
            )
            for w in waits[1:]:
                d2 = self.nc.sync.drain()
                d2.ins.sync_info = mybir.SyncInfo(on_wait=[w], on_update=[])

        self.nc.all_engine_barrier()
        assert self.sems is not None
        popped = self.nc._tile_sem_poison_stack.pop()
        assert popped is self._sem_poison
        self.nc.clear_and_free_semaphores(list(self.sems.allocated().values()))
        self.nc.all_engine_barrier()

    tile.TileContext._drain_and_barrier = _drain_and_barrier
    tile.TileContext._drain_split_patched = True


def _split_multi_waits(nc) -> None:
    """Walrus accepts one sync-wait per instruction: hoist extra waits onto
    single-wait Drain instructions on the same engine, inserted just before."""
    for fn in nc.m.functions:
        for blk in fn.blocks:
            changed = False
            out = []
            for ins in blk.instructions:
                si = ins.sync_info
                if si is not None and len(si.on_wait) > 1:
                    changed = True
                    waits = list(si.on_wait)
                    for j, w in enumerate(waits[:-1]):
                        d = mybir.InstDrain(name=f"{ins.name}-ws{j}")
                        d.engine = ins.engine
                        d.sync_info = mybir.SyncInfo(on_wait=[w], on_update=[])
                        out.append(d)
                    ins.sync_info = mybir.SyncInfo(
                        on_wait=[waits[-1]], on_update=list(si.on_update)
                    )
                out.append(ins)
            if changed:
                blk.instructions = out


# ------------------------------------------------------------- device program
def _host_constants():
    # stats row r = g*16+k; cols: [sums (g',c) 0:256 | sqs (g',c) 256:512 |
    # counts 512].  Keep only the block-diagonal g'==g pieces + counts.
    mask = np.zeros((128, 513), dtype=np.float32)
    for r in range(128):
        g = r // K
        mask[r, g * C:(g + 1) * C] = 1.0
        mask[r, 256 + g * C:256 + (g + 1) * C] = 1.0
        mask[r, 512] = 1.0
    sel = np.zeros((128, K), dtype=np.float32)
    for r in range(128):
        sel[r, r % K] = 1.0
    ident16 = np.eye(16, dtype=np.float32)
    ones_row = np.ones((1, 16), dtype=np.float32)
    # final-combine column: divides the per-label partial losses by K
    ones_col = np.full((16, 1), 1.0 / K, dtype=np.float32)
    # pair mask pre-scaled by the hinge-term 1/(K-1) normalizer
    triu = np.triu(np.ones((K, K), dtype=np.float32), k=1) / (K - 1)
    return mask, sel, ident16, ones_row, ones_col, triu


def _build_kernel():
    _patch_tile_drain()
    nc = bass.Bass("TRN2")

    fpk = nc.dram_tensor("fpk", [128, SW * FC], FP8, kind="ExternalInput")
    ohd = nc.dram_tensor("ohd", [128, SW * OC], FP8, kind="ExternalInput")
    out = nc.dram_tensor("out", [1, 1], FP32, kind="ExternalOutput")
    dbg = (nc.dram_tensor("dbg", [128, 513], FP32, kind="ExternalOutput")
           if DEBUG_STATS else None)

    mask_np, sel_np, id16_np, ones_row_np, ones_col_np, triu_np = \
        _host_constants()
    c_mask = nc.inline_tensor(mask_np, name="c_mask")
    c_sel = nc.inline_tensor(sel_np, name="c_sel")
    c_id16 = nc.inline_tensor(id16_np, name="c_id16")
    c_ones_row = nc.inline_tensor(ones_row_np, name="c_ones_row")
    c_ones_col = nc.inline_tensor(ones_col_np, name="c_ones_col")
    c_triu = nc.inline_tensor(triu_np, name="c_triu")

    DR = mybir.MatmulPerfMode.DoubleRow

    with tile.TileContext(nc) as tc:
        with (
            tc.tile_pool(name="consts", bufs=1) as consts,
            tc.tile_pool(name="feat", bufs=BUFS) as featp,
            tc.tile_pool(name="oh", bufs=BUFS) as ohp,
            tc.tile_pool(name="sq", bufs=BUFS) as sqp,
            tc.tile_pool(name="acc", bufs=1, space="PSUM") as accp,
            tc.tile_pool(name="eps", bufs=1, space="PSUM") as epsp,
            tc.tile_pool(name="epi", bufs=1) as epi,
        ):
            psA = accp.tile([128, 256], FP32)   # one-hot @ features
            psB = accp.tile([128, 257], FP32)   # one-hot @ [features^2 | 1]

            for ci in range(NCHUNK):
                ft = featp.tile([128, CHUNK * FC], FP8)
                nc.sync.dma_start(
                    out=ft, in_=fpk[:, ci * CHUNK * FC:(ci + 1) * CHUNK * FC]
                )
                oh = ohp.tile([128, CHUNK * OC], FP8)
                nc.gpsimd.dma_start(
                    out=oh, in_=ohd[:, ci * CHUNK * OC:(ci + 1) * CHUNK * OC]
                )
                sq = sqp.tile([128, CHUNK * 2 * SQS], FP8)

                ft4 = ft.rearrange("p (w i j) -> p w i j", i=2, j=G * C)
                sq4 = sq.rearrange("p (w i s) -> p w i s", i=2, s=SQS)
                oh4 = oh.rearrange("p (w i m) -> p w i m", i=2, m=G * K)

                # squares: column-split across DVE / ACT (/ Pool), two
                # sub-ops per engine so matmuls unblock at half-chunk
                c1 = DVE_COLS
                c2 = DVE_COLS + ACT_COLS
                HW2 = CHUNK // 2
                for h in range(2):
                    s = slice(h * HW2, (h + 1) * HW2)
                    nc.vector.tensor_mul(
                        sq4[:, s, :, 0:c1], ft4[:, s, :, 0:c1],
                        ft4[:, s, :, 0:c1]
                    )
                    nc.scalar.activation(
                        out=sq4[:, s, :, c1:c2], in_=ft4[:, s, :, c1:c2],
                        func=mybir.ActivationFunctionType.Square,
                    )
                    if POOL_COLS:
                        nc.gpsimd.tensor_mul(
                            sq4[:, s, :, c2:G * C],
                            ft4[:, s, :, c2:G * C], ft4[:, s, :, c2:G * C],
                        )
                nc.vector.memset(sq4[:, :, :, G * C:SQS], 1.0)

                # ---- segment matmuls (DoubleRow: 256-pixel contraction)
                for w in range(CHUNK):
                    gw = ci * CHUNK + w
                    lhsT = oh4[:, w]
                    nc.tensor.matmul(
                        psA[:, :], lhsT, ft4[:, w],
                        start=(gw == 0), stop=(gw == SW - 1), perf_mode=DR,
                    )
                    nc.tensor.matmul(
                        psB[:, :], lhsT, sq4[:, w],
                        start=(gw == 0), stop=(gw == SW - 1), perf_mode=DR,
                    )

            # ---- constants into SBUF (issued after the streaming DMAs so
            # they don't delay the first feature chunk; only the epilogue
            # consumes them)
            sb_mask = consts.tile([128, 513], FP32)
            nc.sync.dma_start(out=sb_mask, in_=c_mask[:, :])
            sb_sel = consts.tile([128, K], FP32)
            nc.sync.dma_start(out=sb_sel, in_=c_sel[:, :])
            sb_id16 = consts.tile([16, 16], FP32)
            nc.sync.dma_start(out=sb_id16, in_=c_id16[:, :])
            sb_ones_row = consts.tile([1, 16], FP32)
            nc.sync.dma_start(out=sb_ones_row, in_=c_ones_row[:, :])
            sb_ones_col = consts.tile([16, 1], FP32)
            nc.sync.dma_start(out=sb_ones_col, in_=c_ones_col[:, :])
            sb_triu = consts.tile([16, 16], FP32)
            nc.sync.dma_start(out=sb_triu, in_=c_triu[:, :])

            # ================= epilogue: stats -> scalar loss =================
            if dbg is not None:
                stats = epi.tile([128, 513], FP32)
                nc.vector.tensor_copy(stats[:, 0:256], psA)
                nc.vector.tensor_copy(stats[:, 256:513], psB)
                nc.sync.dma_start(out=dbg[:, :], in_=stats)

            masked = epi.tile([128, 513], FP32)
            nc.vector.tensor_mul(masked[:, 0:256], psA, sb_mask[:, 0:256])
            nc.vector.tensor_mul(masked[:, 256:513], psB, sb_mask[:, 256:513])

            # fold the 8 group blocks into [16, *] with sel (r -> r%16)
            psum2a = epsp.tile([16, 256], FP32)
            nc.tensor.matmul(psum2a[:, :], sb_sel, masked[:, 0:256],
                             start=True, stop=True)
            psum2b = epsp.tile([16, 257], FP32)
            nc.tensor.matmul(psum2b[:, :], sb_sel, masked[:, 256:513],
                             start=True, stop=True)

            # fold the 8 (g', c) column blocks of 32 down to [16, 32]
            # (DVE may read at most one non-scalar input from PSUM)
            comb_a = epi.tile([16, 128], FP32)
            nc.vector.tensor_copy(comb_a, psum2a[:, 0:128])
            t128 = epi.tile([16, 128], FP32)
            nc.vector.tensor_add(t128, comb_a, psum2a[:, 128:256])
            t64 = epi.tile([16, 64], FP32)
            nc.vector.tensor_add(t64, t128[:, 0:64], t128[:, 64:128])
            sums = epi.tile([16, 32], FP32)
            nc.vector.tensor_add(sums, t64[:, 0:32], t64[:, 32:64])
            comb_b = epi.tile([16, 128], FP32)
            nc.vector.tensor_copy(comb_b, psum2b[:, 0:128])
            u128 = epi.tile([16, 128], FP32)
            nc.vector.tensor_add(u128, comb_b, psum2b[:, 128:256])
            u64 = epi.tile([16, 64], FP32)
            nc.vector.tensor_add(u64, u128[:, 0:64], u128[:, 64:128])
            sqs = epi.tile([16, 32], FP32)
            nc.vector.tensor_add(sqs, u64[:, 0:32], u64[:, 32:64])

            recip = epi.tile([16, 1], FP32)
            nc.vector.reciprocal(out=recip, in_=psum2b[:, 256:257])

            means = epi.tile([16, 32], FP32)
            nc.vector.tensor_scalar_mul(out=means, in0=sums, scalar1=recip)
            msq = epi.tile([16, 32], FP32)
            nc.vector.tensor_mul(msq, means, means)
            m2 = epi.tile([16, 1], FP32)
            nc.vector.tensor_reduce(
                out=m2, in_=msq, axis=mybir.AxisListType.X,
                op=mybir.AluOpType.add,
            )
            sqk = epi.tile([16, 1], FP32)
            nc.vector.tensor_reduce(
                out=sqk, in_=sqs, axis=mybir.AxisListType.X,
                op=mybir.AluOpType.add,
            )
            # vark = sqk/counts - m2 in one op
            vark = epi.tile([16, 1], FP32)
            nc.vector.tensor_scalar(
                out=vark, in0=sqk, scalar1=recip, scalar2=m2,
                op0=mybir.AluOpType.mult, op1=mybir.AluOpType.subtract,
            )

            # pairwise distances: diff2 = m2_i + m2_j - 2 * means @ means.T
            psumT = epsp.tile([32, 16], FP32)
            nc.tensor.transpose(psumT[:, :], means, sb_id16)
            meansT = epi.tile([32, 16], FP32)
            nc.vector.tensor_copy(meansT, psumT)
            meansTn2 = epi.tile([32, 16], FP32)
            nc.vector.tensor_scalar_mul(out=meansTn2, in0=meansT, scalar1=-2.0)

            psumR = epsp.tile([1, 16], FP32)
            nc.tensor.transpose(psumR[:, :], m2, sb_id16)
            m2row = epi.tile([1, 16], FP32)
            nc.vector.tensor_copy(m2row, psumR)

            psumD = epsp.tile([16, 16], FP32)
            nc.tensor.matmul(psumD[:, :], sb_ones_row, m2row,
                             start=True, stop=False)
            nc.tensor.matmul(psumD[:, :], m2row, sb_ones_row,
                             start=False, stop=False)
            nc.tensor.matmul(psumD[:, :], meansTn2, meansT,
                             start=False, stop=True)

            # one ACT sqrt over [diff2 | m2] -> [dist | reg]
            dm = epi.tile([16, 17], FP32)
            nc.vector.tensor_scalar_max(out=dm[:, 0:16], in0=psumD,
                                        scalar1=0.0)
            nc.vector.tensor_copy(dm[:, 16:17], m2)
            dr = epi.tile([16, 17], FP32)
            nc.scalar.activation(out=dr, in_=dm,
                                 func=mybir.ActivationFunctionType.Sqrt)

            hinge = epi.tile([16, 16], FP32)
            nc.vector.tensor_scalar(
                out=hinge, in0=dr[:, 0:16], scalar1=-1.0, scalar2=2.0 * DD,
                op0=mybir.AluOpType.mult, op1=mybir.AluOpType.add,
            )
            nc.vector.tensor_scalar_max(out=hinge, in0=hinge, scalar1=0.0)
            nc.vector.tensor_mul(hinge, hinge, hinge)

            # final [16, 18] = [vark | gamma*reg | hinge * triu/(K-1)];
            # ones_col is pre-scaled by 1/K, so loss = sum(fin)
            final = epi.tile([16, 18], FP32)
            nc.vector.tensor_copy(final[:, 0:1], vark)
            nc.vector.tensor_scalar(
                out=final[:, 1:2], in0=dr[:, 16:17], scalar1=GAMMA,
                scalar2=None, op0=mybir.AluOpType.mult,
            )
            nc.vector.tensor_mul(final[:, 2:18], hinge, sb_triu)

            psumS = epsp.tile([1, 18], FP32)
            nc.tensor.matmul(psumS[:, :], sb_ones_col, final,
                             start=True, stop=True)
            loss = epi.tile([1, 1], FP32)
            nc.vector.tensor_reduce(
                out=loss, in_=psumS, axis=mybir.AxisListType.X,
                op=mybir.AluOpType.add,
            )
            nc.sync.dma_start(out=out[:, :], in_=loss)

    _split_multi_waits(nc)
    return nc


_NC_CACHE = {}


def _get_kernel():
    key = (DEBUG_STATS,)
    if key not in _NC_CACHE:
        _NC_CACHE[key] = _build_kernel()
    return _NC_CACHE[key]


# --------------------------------------------------------------- entry point
def _marshal_image(feat: np.ndarray, lab: np.ndarray):
    # feat [C, H, W] f32 -> fpk [128 p, (w i g c)] fp8;
    # lab [H, W] int -> one-hot ohd [128 p, (w i g k)] fp8.
    # pixel n = g*PG + w*256 + i*128 + p
    f5 = feat.reshape(C, G, SW, 2, 128)
    fpk = np.ascontiguousarray(
        f5.transpose(4, 2, 3, 1, 0).reshape(128, SW * FC)
    ).astype(FP8_NP)
    l4 = lab.reshape(G, SW, 2, 128)
    ohb = (l4[..., None] == np.arange(K, dtype=l4.dtype))
    ohd = np.ascontiguousarray(
        ohb.transpose(3, 1, 2, 0, 4).reshape(128, SW * OC)
    ).astype(FP8_NP)
    return fpk, ohd


def kernel(features_batch, labels_batch, num_instances):
    assert int(num_instances) == K
    features_batch = np.asarray(features_batch, dtype=np.float32)
    labels_batch = np.asarray(labels_batch)
    assert features_batch.shape == (B, C, H, W)

    nc = _get_kernel()
    in_maps = []
    for i in range(B):
        fpk, ohd = _marshal_image(features_batch[i], labels_batch[i])
        in_maps.append({"fpk": fpk, "ohd": ohd})

    res = run_bass_kernel_spmd(
        nc, in_maps, core_ids=list(range(B)), trace=TRACE
    )
    kernel.last_result = res
    losses = [res.results[i]["out"][0, 0] for i in range(B)]
    total = np.float64(0.0)
    for v in losses:
        total += np.float64(v)
    return np.array(total / (B + 1), dtype=np.float32)


# revision 12
# speedup vs baseline: 1.3029x; 1.0223x over previous
"""Trainium2 kernel for nn_ContrasiveLoss (segment-reduce contrastive loss).

Strategy (data-parallel, one image per NeuronCore, 8 cores):
  Per-image loss needs only per-segment statistics
      counts[k], sums[k, c], sqsums[k, c]
  (the variance term telescopes).  Statistics are computed as one-hot
  matmuls on the TensorEngine in fp8-e4m3 DoubleRow mode: each matmul
  contracts 256 pixels (2 k-tiles of 128 partitions) for 8 pixel groups
  at once (8 groups x 16 labels = 128 PSUM partitions).  Per 256-pixel
  super-window there are two accumulating matmuls:
      A: one-hot^T @ features            -> [128, 256]  (bank A)
      B: one-hot^T @ [features^2 | 1]    -> [128, 257]  (bank B)
  Features and the one-hot encoding of the labels are marshaled host-side
  into fp8 with pixels on partitions, so device DMAs are plain contiguous
  copies (no xbar transpose).  Squares are computed on-device (DVE + ACT
  split).  A small epilogue folds the 8 group blocks, computes the
  variance/hinge/regularizer terms and writes one scalar; the host sums
  the 8 scalars and divides by (B+1).
"""

import ml_dtypes
import numpy as np

import concourse.bass as bass
import concourse.mybir as mybir
import concourse.tile as tile
from concourse.bass_utils import run_bass_kernel_spmd
from concourse.vector_clock import ScopedClock

# ---------------------------------------------------------------- problem dims
B, C, H, W = 8, 32, 512, 512
K = 16
G = 8                    # pixel groups; G*K = 128 PSUM partitions
N = H * W                # pixels per image
PG = N // G              # 32768 pixels per group
SW = PG // 256           # 128 super-windows (256 pixels each, per group)
CHUNK = 16               # super-windows per DMA chunk
NCHUNK = SW // CHUNK     # 8
FC = 2 * G * C           # 512 feature cols per super-window
OC = 2 * G * K           # 256 one-hot cols per super-window
SQS = G * C + 1          # 257: squares + ones column (per k-tile)
# squares column split across engines (of the 256 feature columns)
DVE_COLS = 124
ACT_COLS = 92
POOL_COLS = 256 - DVE_COLS - ACT_COLS
BUFS = 5                 # chunk pipeline depth
# chunk sizes in super-windows; first two halved so the PE starts sooner
CHUNKS = [8, 8] + [16] * 7
assert sum(CHUNKS) == SW

DD = 2.5
GAMMA = 0.005

FP8 = mybir.dt.float8e4
FP8_NP = ml_dtypes.float8_e4m3
FP32 = mybir.dt.float32

TRACE = False            # test harness flips this for NTFF profiling
DEBUG_STATS = False      # also emit the raw [128, 513] stats for verification


# ------------------------------------------------- container-specific patches
def _patch_tile_drain() -> None:
    """This container's walrus build accepts only ONE sync-wait command per
    instruction, but TileContext's tail drain attaches one wait per active
    semaphore lane.  Split the tail drain into a chain of single-wait drains.
    """
    if getattr(tile.TileContext, "_drain_split_patched", False):
        return

    def _drain_and_barrier(self, tick_clock, wait_clock):
        drain_inst = self.nc.sync.drain()
        wait_clock.add_sem_waits(
            drain_inst.ins, ScopedClock({None: tick_clock.global_clock})
        )
        si = drain_inst.ins.sync_info
        if si is not None and len(si.on_wait) > 1:
            waits = list(si.on_wait)
            drain_inst.ins.sync_info = mybir.SyncInfo(
                on_wait=[waits[0]], on_update=list(si.on_update)
            )
            for w in waits[1:]:
                d2 = self.nc.sync.drain()
                d2.ins.sync_info = mybir.SyncInfo(on_wait=[w], on_update=[])

        self.nc.all_engine_barrier()
        assert self.sems is not None
        popped = self.nc._tile_sem_poison_stack.pop()
        assert popped is self._sem_poison
        self.nc.clear_and_free_semaphores(list(self.sems.allocated().values()))
        self.nc.all_engine_barrier()

    tile.TileContext._drain_and_barrier = _drain_and_barrier
    tile.TileContext._drain_split_patched = True


def _split_multi_waits(nc) -> None:
    """Walrus accepts one sync-wait per instruction: hoist extra waits onto
    single-wait Drain instructions on the same engine, inserted just before."""
    for fn in nc.m.functions:
        for blk in fn.blocks:
            changed = False
            out = []
            for ins in blk.instructions:
                si = ins.sync_info
                if si is not None and len(si.on_wait) > 1:
                    changed = True
                    waits = list(si.on_wait)
                    for j, w in enumerate(waits[:-1]):
                        d = mybir.InstDrain(name=f"{ins.name}-ws{j}")
                        d.engine = ins.engine
                        d.sync_info = mybir.SyncInfo(on_wait=[w], on_update=[])
                        out.append(d)
                    ins.sync_info = mybir.SyncInfo(
                        on_wait=[waits[-1]], on_update=list(si.on_update)
                    )
                out.append(ins)
            if changed:
                blk.instructions = out


# ------------------------------------------------------------- device program
def _host_constants():
    # stats row r = g*16+k; cols: [sums (g',c) 0:256 | sqs (g',c) 256:512 |
    # counts 512].  Keep only the block-diagonal g'==g pieces + counts.
    mask = np.zeros((128, 513), dtype=np.float32)
    for r in range(128):
        g = r // K
        mask[r, g * C:(g + 1) * C] = 1.0
        mask[r, 256 + g * C:256 + (g + 1) * C] = 1.0
        mask[r, 512] = 1.0
    sel = np.zeros((128, K), dtype=np.float32)
    for r in range(128):
        sel[r, r % K] = 1.0
    ident16 = np.eye(16, dtype=np.float32)
    ones_row = np.ones((1, 16), dtype=np.float32)
    # final-combine column: divides the per-label partial losses by K
    ones_col = np.full((16, 1), 1.0 / K, dtype=np.float32)
    # pair mask pre-scaled by the hinge-term 1/(K-1) normalizer
    triu = np.triu(np.ones((K, K), dtype=np.float32), k=1) / (K - 1)
    return mask, sel, ident16, ones_row, ones_col, triu


def _build_kernel():
    _patch_tile_drain()
    nc = bass.Bass("TRN2")

    fpk = nc.dram_tensor("fpk", [128, SW * FC], FP8, kind="ExternalInput")
    ohd = nc.dram_tensor("ohd", [128, SW * OC], FP8, kind="ExternalInput")
    out = nc.dram_tensor("out", [1, 1], FP32, kind="ExternalOutput")
    dbg = (nc.dram_tensor("dbg", [128, 513], FP32, kind="ExternalOutput")
           if DEBUG_STATS else None)

    mask_np, sel_np, id16_np, ones_row_np, ones_col_np, triu_np = \
        _host_constants()
    c_mask = nc.inline_tensor(mask_np, name="c_mask")
    c_sel = nc.inline_tensor(sel_np, name="c_sel")
    c_id16 = nc.inline_tensor(id16_np, name="c_id16")
    c_ones_row = nc.inline_tensor(ones_row_np, name="c_ones_row")
    c_ones_col = nc.inline_tensor(ones_col_np, name="c_ones_col")
    c_triu = nc.inline_tensor(triu_np, name="c_triu")

    DR = mybir.MatmulPerfMode.DoubleRow

    with tile.TileContext(nc) as tc:
        with (
            tc.tile_pool(name="consts", bufs=1) as consts,
            tc.tile_pool(name="feat", bufs=BUFS) as featp,
            tc.tile_pool(name="oh", bufs=BUFS) as ohp,
            tc.tile_pool(name="sq", bufs=BUFS) as sqp,
            tc.tile_pool(name="acc", bufs=1, space="PSUM") as accp,
            tc.tile_pool(name="eps", bufs=1, space="PSUM") as epsp,
            tc.tile_pool(name="epi", bufs=1) as epi,
        ):
            psA = accp.tile([128, 256], FP32)   # one-hot @ features
            psB = accp.tile([128, 257], FP32)   # one-hot @ [features^2 | 1]

            sw0 = 0
            for n_sw in CHUNKS:
                ft = featp.tile([128, n_sw * FC], FP8)
                nc.sync.dma_start(
                    out=ft, in_=fpk[:, sw0 * FC:(sw0 + n_sw) * FC]
                )
                oh = ohp.tile([128, n_sw * OC], FP8)
                nc.scalar.dma_start(
                    out=oh, in_=ohd[:, sw0 * OC:(sw0 + n_sw) * OC]
                )
                sq = sqp.tile([128, n_sw * 2 * SQS], FP8)

                ft4 = ft.rearrange("p (w i j) -> p w i j", i=2, j=G * C)
                sq4 = sq.rearrange("p (w i s) -> p w i s", i=2, s=SQS)
                oh4 = oh.rearrange("p (w i m) -> p w i m", i=2, m=G * K)

                # squares: column-split across DVE / ACT / Pool, two
                # sub-ops per engine so matmuls unblock at half-chunk
                c1 = DVE_COLS
                c2 = DVE_COLS + ACT_COLS
                HW2 = n_sw // 2
                for h in range(2):
                    s = slice(h * HW2, (h + 1) * HW2)
                    nc.vector.tensor_mul(
                        sq4[:, s, :, 0:c1], ft4[:, s, :, 0:c1],
                        ft4[:, s, :, 0:c1]
                    )
                    nc.scalar.activation(
                        out=sq4[:, s, :, c1:c2], in_=ft4[:, s, :, c1:c2],
                        func=mybir.ActivationFunctionType.Square,
                    )
                    if POOL_COLS:
                        nc.gpsimd.tensor_mul(
                            sq4[:, s, :, c2:G * C],
                            ft4[:, s, :, c2:G * C], ft4[:, s, :, c2:G * C],
                        )
                nc.vector.memset(sq4[:, :, :, G * C:SQS], 1.0)

                # ---- segment matmuls (DoubleRow: 256-pixel contraction)
                for w in range(n_sw):
                    gw = sw0 + w
                    lhsT = oh4[:, w]
                    nc.tensor.matmul(
                        psA[:, :], lhsT, ft4[:, w],
                        start=(gw == 0), stop=(gw == SW - 1), perf_mode=DR,
                    )
                    nc.tensor.matmul(
                        psB[:, :], lhsT, sq4[:, w],
                        start=(gw == 0), stop=(gw == SW - 1), perf_mode=DR,
                    )
                sw0 += n_sw

            # ---- constants into SBUF (issued after the streaming DMAs so
            # they don't delay the first feature chunk; only the epilogue
            # consumes them)
            sb_mask = consts.tile([128, 513], FP32)
            nc.sync.dma_start(out=sb_mask, in_=c_mask[:, :])
            sb_sel = consts.tile([128, K], FP32)
            nc.sync.dma_start(out=sb_sel, in_=c_sel[:, :])
            sb_id16 = consts.tile([16, 16], FP32)
            nc.sync.dma_start(out=sb_id16, in_=c_id16[:, :])
            sb_ones_row = consts.tile([1, 16], FP32)
            nc.sync.dma_start(out=sb_ones_row, in_=c_ones_row[:, :])
            sb_ones_col = consts.tile([16, 1], FP32)
            nc.sync.dma_start(out=sb_ones_col, in_=c_ones_col[:, :])
            sb_triu = consts.tile([16, 16], FP32)
            nc.sync.dma_start(out=sb_triu, in_=c_triu[:, :])

            # ================= epilogue: stats -> scalar loss =================
            if dbg is not None:
                stats = epi.tile([128, 513], FP32)
                nc.vector.tensor_copy(stats[:, 0:256], psA)
                nc.vector.tensor_copy(stats[:, 256:513], psB)
                nc.sync.dma_start(out=dbg[:, :], in_=stats)

            masked = epi.tile([128, 513], FP32)
            nc.vector.tensor_mul(masked[:, 0:256], psA, sb_mask[:, 0:256])
            nc.vector.tensor_mul(masked[:, 256:513], psB, sb_mask[:, 256:513])

            # fold the 8 group blocks into [16, *] with sel (r -> r%16)
            psum2a = epsp.tile([16, 256], FP32)
            nc.tensor.matmul(psum2a[:, :], sb_sel, masked[:, 0:256],
                             start=True, stop=True)
            psum2b = epsp.tile([16, 257], FP32)
            nc.tensor.matmul(psum2b[:, :], sb_sel, masked[:, 256:513],
                             start=True, stop=True)

            # fold the 8 (g', c) column blocks of 32 down to [16, 32]
            # (DVE may read at most one non-scalar input from PSUM)
            comb_a = epi.tile([16, 128], FP32)
            nc.vector.tensor_copy(comb_a, psum2a[:, 0:128])
            t128 = epi.tile([16, 128], FP32)
            nc.vector.tensor_add(t128, comb_a, psum2a[:, 128:256])
            t64 = epi.tile([16, 64], FP32)
            nc.vector.tensor_add(t64, t128[:, 0:64], t128[:, 64:128])
            sums = epi.tile([16, 32], FP32)
            nc.vector.tensor_add(sums, t64[:, 0:32], t64[:, 32:64])
            comb_b = epi.tile([16, 128], FP32)
            nc.vector.tensor_copy(comb_b, psum2b[:, 0:128])
            u128 = epi.tile([16, 128], FP32)
            nc.vector.tensor_add(u128, comb_b, psum2b[:, 128:256])
            u64 = epi.tile([16, 64], FP32)
            nc.vector.tensor_add(u64, u128[:, 0:64], u128[:, 64:128])
            sqs = epi.tile([16, 32], FP32)
            nc.vector.tensor_add(sqs, u64[:, 0:32], u64[:, 32:64])

            recip = epi.tile([16, 1], FP32)
            nc.vector.reciprocal(out=recip, in_=psum2b[:, 256:257])

            means = epi.tile([16, 32], FP32)
            nc.vector.tensor_scalar_mul(out=means, in0=sums, scalar1=recip)
            msq = epi.tile([16, 32], FP32)
            nc.vector.tensor_mul(msq, means, means)
            m2 = epi.tile([16, 1], FP32)
            nc.vector.tensor_reduce(
                out=m2, in_=msq, axis=mybir.AxisListType.X,
                op=mybir.AluOpType.add,
            )
            sqk = epi.tile([16, 1], FP32)
            nc.vector.tensor_reduce(
                out=sqk, in_=sqs, axis=mybir.AxisListType.X,
                op=mybir.AluOpType.add,
            )
            # vark = sqk/counts - m2 in one op
            vark = epi.tile([16, 1], FP32)
            nc.vector.tensor_scalar(
                out=vark, in0=sqk, scalar1=recip, scalar2=m2,
                op0=mybir.AluOpType.mult, op1=mybir.AluOpType.subtract,
            )

            # pairwise distances: diff2 = m2_i + m2_j - 2 * means @ means.T
            psumT = epsp.tile([32, 16], FP32)
            nc.tensor.transpose(psumT[:, :], means, sb_id16)
            meansT = epi.tile([32, 16], FP32)
            nc.vector.tensor_copy(meansT, psumT)
            meansTn2 = epi.tile([32, 16], FP32)
            nc.vector.tensor_scalar_mul(out=meansTn2, in0=meansT, scalar1=-2.0)

            psumR = epsp.tile([1, 16], FP32)
            nc.tensor.transpose(psumR[:, :], m2, sb_id16)
            m2row = epi.tile([1, 16], FP32)
            nc.vector.tensor_copy(m2row, psumR)

            psumD = epsp.tile([16, 16], FP32)
            nc.tensor.matmul(psumD[:, :], sb_ones_row, m2row,
                             start=True, stop=False)
            nc.tensor.matmul(psumD[:, :], m2row, sb_ones_row,
                             start=False, stop=False)
            nc.tensor.matmul(psumD[:, :], meansTn2, meansT,
                             start=False, stop=True)

            # one ACT sqrt over [diff2 | m2] -> [dist | reg]
            dm = epi.tile([16, 17], FP32)
            nc.vector.tensor_scalar_max(out=dm[:, 0:16], in0=psumD,
                                        scalar1=0.0)
            nc.vector.tensor_copy(dm[:, 16:17], m2)
            dr = epi.tile([16, 17], FP32)
            nc.scalar.activation(out=dr, in_=dm,
                                 func=mybir.ActivationFunctionType.Sqrt)

            hinge = epi.tile([16, 16], FP32)
            nc.vector.tensor_scalar(
                out=hinge, in0=dr[:, 0:16], scalar1=-1.0, scalar2=2.0 * DD,
                op0=mybir.AluOpType.mult, op1=mybir.AluOpType.add,
            )
            nc.vector.tensor_scalar_max(out=hinge, in0=hinge, scalar1=0.0)
            nc.vector.tensor_mul(hinge, hinge, hinge)

            # final [16, 18] = [vark | gamma*reg | hinge * triu/(K-1)];
            # ones_col is pre-scaled by 1/K, so loss = sum(fin)
            final = epi.tile([16, 18], FP32)
            nc.vector.tensor_copy(final[:, 0:1], vark)
            nc.vector.tensor_scalar(
                out=final[:, 1:2], in0=dr[:, 16:17], scalar1=GAMMA,
                scalar2=None, op0=mybir.AluOpType.mult,
            )
            nc.vector.tensor_mul(final[:, 2:18], hinge, sb_triu)

            psumS = epsp.tile([1, 18], FP32)
            nc.tensor.matmul(psumS[:, :], sb_ones_col, final,
                             start=True, stop=True)
            loss = epi.tile([1, 1], FP32)
            nc.vector.tensor_reduce(
                out=loss, in_=psumS, axis=mybir.AxisListType.X,
                op=mybir.AluOpType.add,
            )
            nc.sync.dma_start(out=out[:, :], in_=loss)

    _split_multi_waits(nc)
    return nc


_NC_CACHE = {}


def _get_kernel():
    key = (DEBUG_STATS,)
    if key not in _NC_CACHE:
        _NC_CACHE[key] = _build_kernel()
    return _NC_CACHE[key]


# --------------------------------------------------------------- entry point
def _marshal_image(feat: np.ndarray, lab: np.ndarray):
    # feat [C, H, W] f32 -> fpk [128 p, (w i g c)] fp8;
    # lab [H, W] int -> one-hot ohd [128 p, (w i g k)] fp8.
    # pixel n = g*PG + w*256 + i*128 + p
    f5 = feat.reshape(C, G, SW, 2, 128)
    fpk = np.ascontiguousarray(
        f5.transpose(4, 2, 3, 1, 0).reshape(128, SW * FC)
    ).astype(FP8_NP)
    l4 = lab.reshape(G, SW, 2, 128)
    ohb = (l4[..., None] == np.arange(K, dtype=l4.dtype))
    ohd = np.ascontiguousarray(
        ohb.transpose(3, 1, 2, 0, 4).reshape(128, SW * OC)
    ).astype(FP8_NP)
    return fpk, ohd


def kernel(features_batch, labels_batch, num_instances):
    assert int(num_instances) == K
    features_batch = np.asarray(features_batch, dtype=np.float32)
    labels_batch = np.asarray(labels_batch)
    assert features_batch.shape == (B, C, H, W)

    nc = _get_kernel()
    in_maps = []
    for i in range(B):
        fpk, ohd = _marshal_image(features_batch[i], labels_batch[i])
        in_maps.append({"fpk": fpk, "ohd": ohd})

    res = run_bass_kernel_spmd(
        nc, in_maps, core_ids=list(range(B)), trace=TRACE
    )
    kernel.last_result = res
    losses = [res.results[i]["out"][0, 0] for i in range(B)]
    total = np.float64(0.0)
    for v in losses:
        total += np.float64(v)
    return np.array(total / (B + 1), dtype=np.float32)
